# revision 43
# baseline (speedup 1.0000x reference)
"""Trainium2 Bass kernel for nn_Block (dense transformer block, sigmoid attention).

Sharding: 8 cores = 2 (batch) x 4 (query-chunk of 512 tokens).
Host rotates the token axis per core so each core's query chunk is tokens
[0, 512) of its rotated view; K/V are computed over all 2048 (rotated) tokens.
Attention output is invariant to key-token order, so rotation is safe as long
as the coulomb matrix columns are rotated identically.

On-chip layout is feature-major ("F layout"): activations live as x^T with
features on SBUF partitions and tokens on the free axis, so every matmul
contracts along partitions with the weight stationary.

Structure: attention runs in TWO query-half passes (256 queries each). The
proj/LN2/MLP tail for half A is emitted after pass B's attention so the tile
scheduler overlaps it with pass B's sigmoid work (ACT is the bottleneck
engine; the tail is PE/DVE-heavy). Only half B's tail is exposed at the end.

Precision strategy (max rel err ~1.2e-2 < 2e-2): the q@k score path runs in
fp8e4 with MatmulPerfMode.DoubleRow (0.5 cycles/row) — sigmoid's bounded
slope absorbs the quantization noise. q/k projections run DoubleRow from an
fp8 copy of z (made on the idle GPSIMD). Score matmuls pad each head's K=64
contraction with a shared all-zeros K-subtile chunk. w_self folds into w_proj
on the host (y3 = att@w_proj + z@(w_self w_proj)); the folded w_sp rides the
proj PSUM accumulation as two DoubleRow matmuls. Everything else (v, att@v,
proj, MLP) stays bf16 — fp8 there measured ~2e-2 error each. fp8-side weights
are scaled x64 into e4m3's normal range; w_proj (bf16) is scaled x64 too so
the shared proj PSUM descales uniformly.

LayerNorm gains/biases fold into downstream weights on the host. rstd uses
ACT Sqrt + DVE reciprocal so only sqrt/sigmoid/gelu tables load.
"""
import numpy as np
import ml_dtypes
from contextlib import ExitStack

import concourse.bacc as bacc
import concourse.mybir as mybir
import concourse.tile as tile
from concourse.bass_utils import run_bass_kernel_spmd

F32 = mybir.dt.float32
F32R = mybir.dt.float32r
BF16 = mybir.dt.bfloat16
FP8 = mybir.dt.float8e4
AF = mybir.ActivationFunctionType
ALU = mybir.AluOpType
DR = mybir.MatmulPerfMode.DoubleRow

B, T, C, H, D = 2, 2048, 512, 8, 64
TQ = 512          # query tokens per core
HQ = 256          # query half
P = 128
KC = C // P       # 4   C partition-chunks
NT = T // 512     # 4   T tiles of 512
NTK = T // P      # 16  key-token chunks of 128
C4 = 4 * C        # 2048
KC4 = C4 // P     # 16
EPS = 1e-5
N_CORES = 8
WS = 64.0         # host-side fp8/proj weight scale
NP8 = ml_dtypes.float8_e4m3fn

_BUILT = None


def _build():
    nc = bacc.Bacc("TRN2", target_bir_lowering=False, debug=False)

    xT_d = nc.dram_tensor("xT", [P, KC, T], BF16, kind="ExternalInput")
    coulT_d = nc.dram_tensor("coulT", [P, NTK, TQ], BF16, kind="ExternalInput")
    wq_d = nc.dram_tensor("wq", [P, KC, C], FP8, kind="ExternalInput")
    wk_d = nc.dram_tensor("wk", [P, KC, C], FP8, kind="ExternalInput")
    wsp_d = nc.dram_tensor("wsp", [P, KC, C], FP8, kind="ExternalInput")
    wv_d = nc.dram_tensor("wv", [P, KC, C], BF16, kind="ExternalInput")
    wproj_d = nc.dram_tensor("wproj", [P, KC, C], BF16, kind="ExternalInput")
    wfc_d = nc.dram_tensor("wfc", [P, KC, C4], BF16, kind="ExternalInput")
    wfcp_d = nc.dram_tensor("wfcp", [P, KC4, C], BF16, kind="ExternalInput")
    bq_d = nc.dram_tensor("bq", [P, KC], F32, kind="ExternalInput")
    bk_d = nc.dram_tensor("bk", [P, KC], F32, kind="ExternalInput")
    bv_d = nc.dram_tensor("bv", [1, C], F32R, kind="ExternalInput")
    bproj_d = nc.dram_tensor("bproj", [P, KC], F32, kind="ExternalInput")
    bfc_d = nc.dram_tensor("bfc", [P, KC4], F32, kind="ExternalInput")
    bfcp_d = nc.dram_tensor("bfcp", [P, KC], F32, kind="ExternalInput")
    cst_d = nc.dram_tensor("cst", [P, 2], BF16, kind="ExternalInput")  # [1, 1/C]
    onesr_d = nc.dram_tensor("onesr", [1, P], F32R, kind="ExternalInput")
    outT_d = nc.dram_tensor("outT", [P, KC, TQ], F32, kind="ExternalOutput")

    IWS = 1.0 / WS
    SIGS = 0.125 / (WS * WS)

    with tile.TileContext(nc) as tc, ExitStack() as octx:
        cst = octx.enter_context(tc.tile_pool(name="cst", bufs=1))
        lateP = octx.enter_context(tc.tile_pool(name="lateP", bufs=1))
        wfcP = octx.enter_context(tc.tile_pool(name="wfcP", bufs=1))
        wB = octx.enter_context(tc.tile_pool(name="wB", bufs=1))
        zP = octx.enter_context(tc.tile_pool(name="zP", bufs=1))
        qkvP = octx.enter_context(tc.tile_pool(name="qkvP", bufs=1))

        # q/k carry one extra all-zeros chunk: the second DoubleRow K-subtile
        # of the score matmuls (pads each head's K=64 contraction to 128).
        z_sb = zP.tile([P, KC, T], BF16)
        z8_sb = zP.tile([P, KC, T], FP8)
        q_sb = qkvP.tile([P, KC + 1, TQ], FP8)
        k_sb = qkvP.tile([P, KC + 1, T], FP8)
        v_sb = qkvP.tile([P, NTK, C], BF16)
        coul_sb = qkvP.tile([P, NTK, TQ], BF16)
        nc.vector.memset(q_sb[:, KC], 0.0)
        nc.vector.memset(k_sb[:, KC], 0.0)

        # ---- constants / biases (vector queue) -----------------------------
        cst_sb = cst.tile([P, 2], BF16)
        nc.sync.dma_start(cst_sb, cst_d[:, :])
        cm_col = cst_sb[:, 1:2]
        onesr_sb = cst.tile([1, P], F32R)
        nc.sync.dma_start(onesr_sb, onesr_d[:, :])
        eps1 = cst.tile([1, 1], F32)
        nc.vector.memset(eps1, EPS)
        bq_sb = cst.tile([P, KC], F32)
        bk_sb = cst.tile([P, KC], F32)
        bproj_sb = cst.tile([P, KC], F32)
        bfc_sb = cst.tile([P, KC4], F32)
        bfcp_sb = cst.tile([P, KC], F32)
        bv_sb = cst.tile([1, C], F32R)

        wsp_sb = wB.tile([P, KC, C], FP8)
        wproj_sb = wB.tile([P, KC, C], BF16)
        wfc_sb = wfcP.tile([P, KC, C4], BF16)
        wfcp_sb = wfcP.tile([P, KC4, C], BF16)

        with ExitStack() as actx:
            wA = actx.enter_context(tc.tile_pool(name="wA", bufs=1))
            wq_sb = wA.tile([P, KC, C], FP8)
            wk_sb = wA.tile([P, KC, C], FP8)
            wv_sb = wA.tile([P, KC, C], BF16)
            # early weights + coulomb on the gpsimd queue (it is idle early);
            # late-phase weights go on the sync queue after the x tiles so
            # the gpsimd queue stays clear for the per-tile z8 copies.
            for sb, d in ((wq_sb, wq_d), (wk_sb, wk_d), (wv_sb, wv_d)):
                nc.gpsimd.dma_start(sb, d[:, :])
            nc.gpsimd.dma_start(coul_sb, coulT_d[:, :])

            # ======= Phase 1: LayerNorm 1 + q/k/v, per 512-token tile =======
            with tc.tile_pool(name="lnX", bufs=6) as lnX, \
                 tc.tile_pool(name="lnR", bufs=8) as lnR, \
                 tc.tile_pool(name="lnS", bufs=4) as lnS, \
                 tc.tile_pool(name="lnZ", bufs=2) as lnZ, \
                 tc.tile_pool(name="psLN", bufs=2, space="PSUM") as psLN, \
                 tc.tile_pool(name="psMM", bufs=2, space="PSUM") as psMM:
                x_tiles = {}
                for n in range(NT):
                    xt = lnX.tile([P, KC, 512], BF16, tag="xt", name=f"xt_{n}")
                    nc.sync.dma_start(xt, xT_d[:, :, n * 512:(n + 1) * 512])
                    x_tiles[n] = xt
                for sb, d in ((bq_sb, bq_d), (bk_sb, bk_d),
                              (bproj_sb, bproj_d), (bfc_sb, bfc_d),
                              (bfcp_sb, bfcp_d)):
                    nc.sync.dma_start(sb, d[:, :])
                nc.sync.dma_start(bv_sb, bv_d[:, :])
                for sb, d in ((wsp_sb, wsp_d), (wproj_sb, wproj_d),
                              (wfc_sb, wfc_d), (wfcp_sb, wfcp_d)):
                    nc.sync.dma_start(sb, d[:, :])
                for n in range(NT):
                    sl = slice(n * 512, (n + 1) * 512)
                    xt = x_tiles[n]
                    # uncentered variance: var = E[x^2] - mean^2 (row math)
                    sq_t = lnS.tile([P, KC, 512], BF16, tag="sq", name=f"sq{n}")
                    nc.vector.tensor_tensor(out=sq_t, in0=xt, in1=xt,
                                            op=ALU.mult)
                    ps_m = psLN.tile([1, 512], F32, tag="st")
                    for kc in range(KC):
                        nc.tensor.matmul(ps_m, lhsT=cm_col, rhs=xt[:, kc],
                                         start=(kc == 0), stop=(kc == KC - 1))
                    m_row = lnR.tile([1, 512], BF16, tag="row", name=f"mrow{n}")
                    with nc.allow_low_precision(reason="LN mean row"):
                        nc.scalar.activation(m_row, ps_m, AF.Copy)
                    m_bc = lnZ.tile([P, 512], BF16, tag="mbc", name=f"mbc{n}")
                    nc.gpsimd.partition_broadcast(m_bc, m_row)
                    ps_v = psLN.tile([1, 512], F32, tag="st")
                    for kc in range(KC):
                        nc.tensor.matmul(ps_v, lhsT=cm_col, rhs=sq_t[:, kc],
                                         start=(kc == 0), stop=(kc == KC - 1))
                    msq_row = lnR.tile([1, 512], F32, tag="row", name=f"msqrow{n}")
                    nc.scalar.square(msq_row, m_row)
                    v_row = lnR.tile([1, 512], F32, tag="row", name=f"vrow{n}")
                    nc.vector.tensor_tensor(out=v_row, in0=ps_v, in1=msq_row,
                                            op=ALU.subtract)
                    zc = lnZ.tile([P, KC, 512], BF16, tag="zc", name=f"zc{n}")
                    nc.vector.tensor_tensor(
                        out=zc, in0=xt,
                        in1=m_bc[:, None, :].to_broadcast([P, KC, 512]),
                        op=ALU.subtract)
                    std_row = lnR.tile([1, 512], F32, tag="row", name=f"sdrow{n}")
                    nc.scalar.activation(std_row, v_row, AF.Sqrt, bias=eps1)
                    rs_row = lnR.tile([1, 512], BF16, tag="row", name=f"rsrow{n}")
                    with nc.allow_low_precision(reason="rstd row"):
                        nc.vector.reciprocal(rs_row, std_row)
                    rs_bc = lnZ.tile([P, 512], BF16, tag="rbc", name=f"rbc{n}")
                    nc.gpsimd.partition_broadcast(rs_bc, rs_row)
                    nc.vector.tensor_tensor(
                        out=z_sb[:, :, sl], in0=zc,
                        in1=rs_bc[:, None, :].to_broadcast([P, KC, 512]),
                        op=ALU.mult)
                    # fp8 copy of z for the q/k/self-fold DoubleRow matmuls;
                    # runs on the otherwise-idle GPSIMD engine.
                    nc.gpsimd.tensor_copy(z8_sb[:, :, sl], z_sb[:, :, sl])

                    # ---- q/k (fp8 DR) and v (bf16) for this token tile -----
                    if n == 0:
                        for mo in range(KC):
                            ps = psMM.tile([P, 512], F32, tag="mm")
                            for kp in range(2):
                                nc.tensor.matmul(
                                    ps,
                                    lhsT=wq_sb[:, 2 * kp:2 * kp + 2,
                                               mo * P:(mo + 1) * P],
                                    rhs=z8_sb[:, 2 * kp:2 * kp + 2, 0:TQ],
                                    start=(kp == 0), stop=(kp == 1),
                                    perf_mode=DR)
                            if mo < 2:
                                nc.scalar.activation(q_sb[:, mo], ps, AF.Identity,
                                                     bias=bq_sb[:, mo:mo + 1])
                            else:
                                nc.vector.tensor_scalar(q_sb[:, mo], ps,
                                                        bq_sb[:, mo:mo + 1],
                                                        None, ALU.add)
                    for mo in range(KC):
                        ps = psMM.tile([P, 512], F32, tag="mm")
                        for kp in range(2):
                            nc.tensor.matmul(
                                ps,
                                lhsT=wk_sb[:, 2 * kp:2 * kp + 2,
                                           mo * P:(mo + 1) * P],
                                rhs=z8_sb[:, 2 * kp:2 * kp + 2, sl],
                                start=(kp == 0), stop=(kp == 1),
                                perf_mode=DR)
                        if mo < 2:
                            nc.scalar.activation(k_sb[:, mo, sl], ps, AF.Identity,
                                                 bias=bk_sb[:, mo:mo + 1])
                        else:
                            nc.vector.tensor_scalar(k_sb[:, mo, sl], ps,
                                                    bk_sb[:, mo:mo + 1],
                                                    None, ALU.add)
                    for ts_ in range(4 * n, 4 * n + 4):
                        ps = psMM.tile([P, 512], F32, tag="mm")
                        for kc in range(KC):
                            nc.tensor.matmul(
                                ps,
                                lhsT=z_sb[:, kc, ts_ * P:(ts_ + 1) * P],
                                rhs=wv_sb[:, kc],
                                start=(kc == 0), stop=False)
                        nc.tensor.matmul(ps, lhsT=onesr_sb, rhs=bv_sb,
                                         start=False, stop=True,
                                         skip_group_check=True)
                        if ts_ % 2 == 0:
                            nc.vector.tensor_copy(v_sb[:, ts_], ps)
                        else:
                            nc.scalar.activation(v_sb[:, ts_], ps, AF.Identity)

        # ======= Attention + per-half tails =================================
        y2 = lateP.tile([P, KC, TQ], BF16, tag="mid_a")
        y3 = lateP.tile([P, KC, TQ], BF16, tag="mid_b")
        z2 = lateP.tile([P, KC, TQ], BF16, tag="z2")

        def attention_pass(hg, hq, psSC, attS, y_ph, filler=None):
            """heads [4*hg, 4*hg+4), queries [hq*HQ, hq*HQ+HQ).

            Each pass holds 6 PSUM banks (scores 2x2, y accumulators 2),
            leaving 2 banks for a concurrently-scheduled tail stage.
            Score PSUM tiles are [P, 2, 512] so each head's accumulation
            group gets a full bank (matmul start zeroes 2KB regions).
            """
            qs = slice(hq * HQ, (hq + 1) * HQ)
            for tkc in range(NTK):
                coul_t = coul_sb[:, tkc, qs]
                s_t = attS.tile([P, 4, HQ], BF16, tag="st")
                for quarter in range(2):
                    sc_ps = psSC.tile([P, 2, 512], F32, tag="sc")
                    for hh in range(2):
                        h = 4 * hg + quarter * 2 + hh
                        chk, po = h // 2, 64 * (h % 2)
                        ck = slice(chk, KC + 1, KC - chk)
                        nc.tensor.matmul(
                            sc_ps[:, hh, 0:HQ],
                            lhsT=k_sb[po:po + 64, ck,
                                      tkc * P:(tkc + 1) * P],
                            rhs=q_sb[po:po + 64, ck, qs],
                            start=True, stop=True, perf_mode=DR)
                    nc.scalar.activation(
                        s_t[:, quarter * 2:quarter * 2 + 2, :],
                        sc_ps[:, :, 0:HQ], AF.Sigmoid, scale=SIGS)
                nc.vector.tensor_tensor(
                    out=s_t, in0=s_t,
                    in1=coul_t[:, None, :].to_broadcast([P, 4, HQ]),
                    op=ALU.mult)
                for hh in range(4):
                    h = 4 * hg + hh
                    po = 64 * (h % 2)
                    nc.tensor.matmul(
                        y_ph[hh // 2][po:po + 64, :],
                        lhsT=v_sb[:, tkc, 64 * h:64 * h + 64],
                        rhs=s_t[:, hh, :],
                        start=(tkc == 0), stop=(tkc == NTK - 1),
                        tile_position=(0, po))
                if filler is not None:
                    for _ in range(2):
                        next(filler, None)

        def tail(hq):
            """proj + LN2 + MLP + out DMA for one query half.

            Uses at most 2 PSUM banks at any time so it can run concurrently
            with an attention pass (6 banks).
            """
            qs = slice(hq * HQ, (hq + 1) * HQ)
            y2h = y2[:, :, qs]
            y3h = y3[:, :, qs]
            z2h = z2[:, :, qs]
            with tc.tile_pool(name=f"psP5{hq}", bufs=2, space="PSUM") as psP5:
                for j in range(KC):
                    ps = psP5.tile([P, HQ], F32, tag="mm")
                    for kc in range(KC):
                        nc.tensor.matmul(
                            ps, lhsT=wproj_sb[:, kc, j * P:(j + 1) * P],
                            rhs=y2h[:, kc],
                            start=(kc == 0), stop=False)
                    for kp in range(2):
                        nc.tensor.matmul(
                            ps,
                            lhsT=wsp_sb[:, 2 * kp:2 * kp + 2, j * P:(j + 1) * P],
                            rhs=z8_sb[:, 2 * kp:2 * kp + 2, qs],
                            start=False, stop=(kp == 1), perf_mode=DR,
                            skip_group_check=True)
                    if j % 2 == 0:
                        nc.vector.tensor_scalar(y3h[:, j], ps, IWS,
                                                bproj_sb[:, j:j + 1], ALU.mult,
                                                ALU.add)
                    else:
                        nc.scalar.activation(y3h[:, j], ps, AF.Identity,
                                             bias=bproj_sb[:, j:j + 1],
                                             scale=IWS)
                    yield

            with tc.tile_pool(name=f"ln2R{hq}", bufs=6) as ln2R, \
                 tc.tile_pool(name=f"ln2S{hq}", bufs=1) as ln2S, \
                 tc.tile_pool(name=f"ln2T{hq}", bufs=2) as ln2T, \
                 tc.tile_pool(name=f"psL2{hq}", bufs=1, space="PSUM") as psLN2:
                sq2 = ln2S.tile([P, KC, HQ], BF16, tag="sq2")
                nc.vector.tensor_tensor(out=sq2, in0=y3h, in1=y3h, op=ALU.mult)
                ps_m2 = psLN2.tile([1, HQ], F32, tag="st2")
                for kc in range(KC):
                    nc.tensor.matmul(ps_m2, lhsT=cm_col, rhs=y3h[:, kc],
                                     start=(kc == 0), stop=(kc == KC - 1))
                m2_row = ln2R.tile([1, HQ], BF16, tag="row2")
                with nc.allow_low_precision(reason="LN mean row"):
                    nc.vector.tensor_copy(m2_row, ps_m2)
                m2_bc = ln2T.tile([P, HQ], BF16, tag="m2bc")
                nc.gpsimd.partition_broadcast(m2_bc, m2_row)
                yield
                ps_v2 = psLN2.tile([1, HQ], F32, tag="st2")
                for kc in range(KC):
                    nc.tensor.matmul(ps_v2, lhsT=cm_col, rhs=sq2[:, kc],
                                     start=(kc == 0), stop=(kc == KC - 1))
                msq2_row = ln2R.tile([1, HQ], F32, tag="row2")
                nc.scalar.square(msq2_row, m2_row)
                v2_row = ln2R.tile([1, HQ], F32, tag="row2")
                nc.vector.tensor_tensor(out=v2_row, in0=ps_v2, in1=msq2_row,
                                        op=ALU.subtract)
                zc2 = ln2T.tile([P, KC, HQ], BF16, tag="zc2")
                nc.vector.tensor_tensor(
                    out=zc2, in0=y3h,
                    in1=m2_bc[:, None, :].to_broadcast([P, KC, HQ]),
                    op=ALU.subtract)
                yield
                sd2_row = ln2R.tile([1, HQ], F32, tag="row2")
                nc.scalar.activation(sd2_row, v2_row, AF.Sqrt, bias=eps1)
                rs2_row = ln2R.tile([1, HQ], BF16, tag="row2")
                with nc.allow_low_precision(reason="rstd row"):
                    nc.vector.reciprocal(rs2_row, sd2_row)
                rs2_bc = ln2T.tile([P, HQ], BF16, tag="r2bc")
                nc.gpsimd.partition_broadcast(rs2_bc, rs2_row)
                nc.vector.tensor_tensor(
                    out=z2h, in0=zc2,
                    in1=rs2_bc[:, None, :].to_broadcast([P, KC, HQ]),
                    op=ALU.mult)
                yield

            with tc.tile_pool(name=f"gP{hq}", bufs=1) as gP:
                g_sb = gP.tile([P, KC4, HQ], BF16)
                out_sb = gP.tile([P, KC, HQ], F32)
                with tc.tile_pool(name=f"psML{hq}", bufs=2,
                                  space="PSUM") as psMLP:
                    for mo in range(KC4):
                        ps = psMLP.tile([P, HQ], F32, tag="mm")
                        for kc in range(KC):
                            nc.tensor.matmul(
                                ps, lhsT=wfc_sb[:, kc, mo * P:(mo + 1) * P],
                                rhs=z2h[:, kc],
                                start=(kc == 0), stop=(kc == KC - 1))
                        nc.scalar.activation(g_sb[:, mo], ps, AF.Gelu,
                                             bias=bfc_sb[:, mo:mo + 1])
                        if mo % 2 == 1:
                            yield
                with tc.tile_pool(name=f"psOJ{hq}", bufs=2,
                                  space="PSUM") as psOJ:
                    for j in range(KC):
                        oj = psOJ.tile([P, HQ], F32, tag="oj")
                        for mo in range(KC4):
                            nc.tensor.matmul(
                                oj, lhsT=wfcp_sb[:, mo, j * P:(j + 1) * P],
                                rhs=g_sb[:, mo],
                                start=(mo == 0), stop=(mo == KC4 - 1))
                        if j % 2 == 0:
                            nc.vector.tensor_scalar(out_sb[:, j], oj,
                                                    bfcp_sb[:, j:j + 1], None,
                                                    ALU.add)
                        else:
                            nc.scalar.activation(out_sb[:, j], oj, AF.Identity,
                                                 bias=bfcp_sb[:, j:j + 1])
                        nc.sync.dma_start(outT_d[:, j, qs], out_sb[:, j])
                        yield

        # Pass order: (heads 0-3, qA), (heads 4-7, qA), (heads 0-3, qB),
        # (heads 4-7, qB). tail(qA) is emitted before the last pass so the
        # scheduler runs it under pass 4's sigmoid wall; only tail(qB) is
        # exposed at the end.
        with tc.tile_pool(name="attS", bufs=3) as attS, \
             tc.tile_pool(name="psSC", bufs=2, space="PSUM") as psSC:
            def one_pass(hg, hq, filler=None):
                qs = slice(hq * HQ, (hq + 1) * HQ)
                with tc.tile_pool(name=f"psY{hg}{hq}", bufs=1,
                                  space="PSUM") as psY:
                    y_ph = [psY.tile([P, HQ], F32, tag=f"y{jj}",
                                     name=f"y_{hg}{hq}_{jj}")
                            for jj in range(2)]
                    attention_pass(hg, hq, psSC, attS, y_ph, filler=filler)
                    for jj in range(2):
                        j = 2 * hg + jj
                        if jj == 0:
                            nc.vector.tensor_copy(y2[:, j, qs], y_ph[jj])
                        else:
                            nc.scalar.activation(y2[:, j, qs], y_ph[jj],
                                                 AF.Identity)

            one_pass(0, 0)
            one_pass(1, 0)
            tail_a = tail(0)
            one_pass(0, 1, filler=tail_a)
            one_pass(1, 1, filler=tail_a)
            for _ in tail_a:
                pass
        for _ in tail(1):
            pass

    nc.compile()
    return nc


def _get_nc():
    global _BUILT
    if _BUILT is None:
        _BUILT = _build()
    return _BUILT


def _fmt_lhs(w):
    """[Cin, Cout] -> [128, Cin//128, Cout] partition-major lhsT layout."""
    return np.ascontiguousarray(
        w.reshape(w.shape[0] // P, P, w.shape[1]).transpose(1, 0, 2))


def _fmt_bias(b):
    """[O] -> [128, O//128] per-partition layout."""
    return np.ascontiguousarray(b.reshape(-1, P).T)


def _prep(inputs):
    f32 = np.float32
    x = np.asarray(inputs["x"], f32)
    coul = np.asarray(inputs["coulomb_matrix"], f32)
    g1 = np.asarray(inputs["ln1_g"], f32)
    b1 = np.asarray(inputs["ln1_b"], f32)
    g2 = np.asarray(inputs["ln2_g"], f32)
    b2 = np.asarray(inputs["ln2_b"], f32)
    wattn = np.asarray(inputs["w_attn"], f32)
    battn = np.asarray(inputs["b_attn"], f32)
    w_self = np.asarray(inputs["w_self"], f32)
    b_self = np.asarray(inputs["b_self"], f32)
    w_proj = np.asarray(inputs["w_proj"], f32)
    b_proj = np.asarray(inputs["b_proj"], f32)
    w_fc = np.asarray(inputs["w_fc"], f32)
    b_fc = np.asarray(inputs["b_fc"], f32)
    w_fcp = np.asarray(inputs["w_fc_proj"], f32)
    b_fcp = np.asarray(inputs["b_fc_proj"], f32)

    wq, wk, wv = wattn[:, 0:C], wattn[:, C:2 * C], wattn[:, 2 * C:]
    # self branch folded into proj: y3 = att@wproj + z@(g1*wself@wproj) + bp'
    wsp = (g1[:, None] * w_self) @ w_proj
    bproj_f = (b_self + b1 @ w_self) @ w_proj + b_proj

    shared = {
        "wq": _fmt_lhs(g1[:, None] * wq * WS).astype(NP8),
        "wk": _fmt_lhs(g1[:, None] * wk * WS).astype(NP8),
        "wsp": _fmt_lhs(wsp * WS).astype(NP8),
        "wv": _fmt_lhs(g1[:, None] * wv).astype(ml_dtypes.bfloat16),
        "wproj": _fmt_lhs(w_proj * WS).astype(ml_dtypes.bfloat16),
        "wfc": _fmt_lhs(g2[:, None] * w_fc).astype(ml_dtypes.bfloat16),
        "wfcp": _fmt_lhs(w_fcp).astype(ml_dtypes.bfloat16),
        "bq": _fmt_bias((battn[0:C] + b1 @ wq) * WS),
        "bk": _fmt_bias((battn[C:2 * C] + b1 @ wk) * WS),
        "bv": (battn[2 * C:] + b1 @ wv).reshape(1, C),
        "bproj": _fmt_bias(bproj_f),
        "bfc": _fmt_bias(b_fc + b2 @ w_fc),
        "bfcp": _fmt_bias(b_fcp),
        "cst": np.stack([np.ones(P, f32), np.full(P, 1.0 / C, f32)],
                        axis=1).astype(ml_dtypes.bfloat16),
        "onesr": np.ones((1, P), f32),
    }
    in_maps = []
    for core in range(N_CORES):
        b, tqi = divmod(core, 4)
        tq0 = tqi * TQ
        xr = np.roll(x[b], -tq0, axis=0)                      # [T, C]
        xT = np.ascontiguousarray(
            xr.T.reshape(KC, P, T).transpose(1, 0, 2)).astype(
                ml_dtypes.bfloat16)                           # [P, KC, T]
        cr = np.roll(coul[b], -tq0, axis=1)[tq0:tq0 + TQ, :]  # [TQ, T]
        coulT = np.ascontiguousarray(
            cr.T.reshape(NTK, P, TQ).transpose(1, 0, 2)).astype(
                ml_dtypes.bfloat16)                           # [P, NTK, TQ]
        m = dict(shared)
        m["xT"] = xT
        m["coulT"] = coulT
        in_maps.append(m)
    return in_maps


def _assemble(results):
    out = np.empty((B, T, C), np.float32)
    for core in range(N_CORES):
        b, tqi = divmod(core, 4)
        tq0 = tqi * TQ
        r = results[core]["outT"]                  # [P, KC, TQ]
        o = r.transpose(1, 0, 2).reshape(C, TQ).T  # [TQ, C]
        out[b, tq0:tq0 + TQ] = o
    return out


def _run(inputs, trace=False):
    nc = _get_nc()
    in_maps = _prep(inputs)
    res = run_bass_kernel_spmd(nc, in_maps, core_ids=list(range(N_CORES)),
                               trace=trace)
    return _assemble(res.results), res


def kernel(**inputs):
    out, _ = _run(inputs)
    return out


# revision 48
# speedup vs baseline: 1.0983x; 1.0983x over previous
"""Trainium2 Bass kernel for nn_Block (dense transformer block, sigmoid attention).

Sharding: 8 cores = 2 (batch) x 4 (query-chunk of 512 tokens).
Host rotates the token axis per core so each core's query chunk is tokens
[0, 512) of its rotated view; K/V are computed over all 2048 (rotated) tokens.
Attention output is invariant to key-token order, so rotation is safe as long
as the coulomb matrix columns are rotated identically.

On-chip layout is feature-major ("F layout"): activations live as x^T with
features on SBUF partitions and tokens on the free axis, so every matmul
contracts along partitions with the weight stationary.

Structure: attention runs in TWO query-half passes (256 queries each). The
proj/LN2/MLP tail for half A is emitted after pass B's attention so the tile
scheduler overlaps it with pass B's sigmoid work (ACT is the bottleneck
engine; the tail is PE/DVE-heavy). Only half B's tail is exposed at the end.

Precision strategy (max rel err ~1.2e-2 < 2e-2): the q@k score path runs in
fp8e4 with MatmulPerfMode.DoubleRow (0.5 cycles/row) — sigmoid's bounded
slope absorbs the quantization noise. q/k projections run DoubleRow from an
fp8 copy of z (made on the idle GPSIMD). Score matmuls pad each head's K=64
contraction with a shared all-zeros K-subtile chunk. w_self folds into w_proj
on the host (y3 = att@w_proj + z@(w_self w_proj)); the folded w_sp rides the
proj PSUM accumulation as two DoubleRow matmuls. Everything else (v, att@v,
proj, MLP) stays bf16 — fp8 there measured ~2e-2 error each. fp8-side weights
are scaled x64 into e4m3's normal range; w_proj (bf16) is scaled x64 too so
the shared proj PSUM descales uniformly.

LayerNorm gains/biases fold into downstream weights on the host. rstd uses
ACT Sqrt + DVE reciprocal so only sqrt/sigmoid/gelu tables load.
"""
import numpy as np
import ml_dtypes
from contextlib import ExitStack

import concourse.bacc as bacc
import concourse.mybir as mybir
import concourse.tile as tile
from concourse.bass_utils import run_bass_kernel_spmd

F32 = mybir.dt.float32
F32R = mybir.dt.float32r
BF16 = mybir.dt.bfloat16
FP8 = mybir.dt.float8e4
AF = mybir.ActivationFunctionType
ALU = mybir.AluOpType
DR = mybir.MatmulPerfMode.DoubleRow

B, T, C, H, D = 2, 2048, 512, 8, 64
TQ = 512          # query tokens per core
HQ = 256          # query half
P = 128
KC = C // P       # 4   C partition-chunks
NT = T // 512     # 4   T tiles of 512
NTK = T // P      # 16  key-token chunks of 128
C4 = 4 * C        # 2048
KC4 = C4 // P     # 16
EPS = 1e-5
N_CORES = 8
WS = 64.0         # host-side fp8/proj weight scale
NP8 = ml_dtypes.float8_e4m3fn

_BUILT = None


def _build():
    nc = bacc.Bacc("TRN2", target_bir_lowering=False, debug=False)

    xT_d = nc.dram_tensor("xT", [P, KC, T], BF16, kind="ExternalInput")
    coulT_d = nc.dram_tensor("coulT", [P, NTK, TQ], BF16, kind="ExternalInput")
    wq_d = nc.dram_tensor("wq", [P, KC, C], FP8, kind="ExternalInput")
    wk_d = nc.dram_tensor("wk", [P, KC, C], FP8, kind="ExternalInput")
    wsp_d = nc.dram_tensor("wsp", [P, KC, C], FP8, kind="ExternalInput")
    wv_d = nc.dram_tensor("wv", [P, KC, C], BF16, kind="ExternalInput")
    wproj_d = nc.dram_tensor("wproj", [P, KC, C], BF16, kind="ExternalInput")
    wfc_d = nc.dram_tensor("wfc", [P, KC, C4], BF16, kind="ExternalInput")
    wfcp_d = nc.dram_tensor("wfcp", [P, KC4, C], BF16, kind="ExternalInput")
    bq_d = nc.dram_tensor("bq", [P, KC], F32, kind="ExternalInput")
    bk_d = nc.dram_tensor("bk", [P, KC], F32, kind="ExternalInput")
    bv_d = nc.dram_tensor("bv", [1, C], F32R, kind="ExternalInput")
    bproj_d = nc.dram_tensor("bproj", [P, KC], F32, kind="ExternalInput")
    bfc_d = nc.dram_tensor("bfc", [P, KC4], F32, kind="ExternalInput")
    bfce_d = nc.dram_tensor("bfce", [P, KC4], F32, kind="ExternalInput")
    bfch_d = nc.dram_tensor("bfch", [P, KC4], F32, kind="ExternalInput")
    bfcp_d = nc.dram_tensor("bfcp", [P, KC], F32, kind="ExternalInput")
    cst_d = nc.dram_tensor("cst", [P, 2], BF16, kind="ExternalInput")  # [1, 1/C]
    onesr_d = nc.dram_tensor("onesr", [1, P], F32R, kind="ExternalInput")
    outT_d = nc.dram_tensor("outT", [P, KC, TQ], F32, kind="ExternalOutput")

    IWS = 1.0 / WS
    SIGS = 0.125 / (WS * WS)

    with tile.TileContext(nc) as tc, ExitStack() as octx:
        cst = octx.enter_context(tc.tile_pool(name="cst", bufs=1))
        lateP = octx.enter_context(tc.tile_pool(name="lateP", bufs=1))
        wfcP = octx.enter_context(tc.tile_pool(name="wfcP", bufs=1))
        wB = octx.enter_context(tc.tile_pool(name="wB", bufs=1))
        zP = octx.enter_context(tc.tile_pool(name="zP", bufs=1))
        qkvP = octx.enter_context(tc.tile_pool(name="qkvP", bufs=1))

        # q/k carry one extra all-zeros chunk: the second DoubleRow K-subtile
        # of the score matmuls (pads each head's K=64 contraction to 128).
        z_sb = zP.tile([P, KC, T], BF16)
        z8_sb = zP.tile([P, KC, T], FP8)
        q_sb = qkvP.tile([P, KC + 1, TQ], FP8)
        k_sb = qkvP.tile([P, KC + 1, T], FP8)
        v_sb = qkvP.tile([P, NTK, C], BF16)
        coul_sb = qkvP.tile([P, NTK, TQ], BF16)
        nc.vector.memset(q_sb[:, KC], 0.0)
        nc.vector.memset(k_sb[:, KC], 0.0)

        # ---- constants / biases (vector queue) -----------------------------
        cst_sb = cst.tile([P, 2], BF16)
        nc.sync.dma_start(cst_sb, cst_d[:, :])
        cm_col = cst_sb[:, 1:2]
        onesr_sb = cst.tile([1, P], F32R)
        nc.sync.dma_start(onesr_sb, onesr_d[:, :])
        eps1 = cst.tile([1, 1], F32)
        nc.vector.memset(eps1, EPS)
        bq_sb = cst.tile([P, KC], F32)
        bk_sb = cst.tile([P, KC], F32)
        bproj_sb = cst.tile([P, KC], F32)
        bfc_sb = cst.tile([P, KC4], F32)
        bfce_sb = cst.tile([P, KC4], F32)
        bfch_sb = cst.tile([P, KC4], F32)
        bfcp_sb = cst.tile([P, KC], F32)
        bv_sb = cst.tile([1, C], F32R)

        wsp_sb = wB.tile([P, KC, C], FP8)
        wproj_sb = wB.tile([P, KC, C], BF16)
        wfc_sb = wfcP.tile([P, KC, C4], BF16)
        wfcp_sb = wfcP.tile([P, KC4, C], BF16)

        with ExitStack() as actx:
            wA = actx.enter_context(tc.tile_pool(name="wA", bufs=1))
            wq_sb = wA.tile([P, KC, C], FP8)
            wk_sb = wA.tile([P, KC, C], FP8)
            wv_sb = wA.tile([P, KC, C], BF16)
            # early weights + coulomb on the gpsimd queue (it is idle early);
            # late-phase weights go on the sync queue after the x tiles so
            # the gpsimd queue stays clear for the per-tile z8 copies.
            for sb, d in ((wq_sb, wq_d), (wk_sb, wk_d), (wv_sb, wv_d)):
                nc.gpsimd.dma_start(sb, d[:, :])
            nc.gpsimd.dma_start(coul_sb, coulT_d[:, :])

            # ======= Phase 1: LayerNorm 1 + q/k/v, per 512-token tile =======
            with tc.tile_pool(name="lnX", bufs=6) as lnX, \
                 tc.tile_pool(name="lnR", bufs=8) as lnR, \
                 tc.tile_pool(name="lnS", bufs=4) as lnS, \
                 tc.tile_pool(name="lnZ", bufs=2) as lnZ, \
                 tc.tile_pool(name="psLN", bufs=2, space="PSUM") as psLN, \
                 tc.tile_pool(name="psMM", bufs=2, space="PSUM") as psMM:
                x_tiles = {}
                for n in range(NT):
                    xt = lnX.tile([P, KC, 512], BF16, tag="xt", name=f"xt_{n}")
                    nc.sync.dma_start(xt, xT_d[:, :, n * 512:(n + 1) * 512])
                    x_tiles[n] = xt
                for sb, d in ((bq_sb, bq_d), (bk_sb, bk_d),
                              (bproj_sb, bproj_d), (bfc_sb, bfc_d),
                              (bfce_sb, bfce_d), (bfch_sb, bfch_d),
                              (bfcp_sb, bfcp_d)):
                    nc.sync.dma_start(sb, d[:, :])
                nc.sync.dma_start(bv_sb, bv_d[:, :])
                for sb, d in ((wsp_sb, wsp_d), (wproj_sb, wproj_d),
                              (wfc_sb, wfc_d), (wfcp_sb, wfcp_d)):
                    nc.sync.dma_start(sb, d[:, :])
                for n in range(NT):
                    sl = slice(n * 512, (n + 1) * 512)
                    xt = x_tiles[n]
                    # uncentered variance: var = E[x^2] - mean^2 (row math)
                    sq_t = lnS.tile([P, KC, 512], BF16, tag="sq", name=f"sq{n}")
                    nc.vector.tensor_tensor(out=sq_t, in0=xt, in1=xt,
                                            op=ALU.mult)
                    ps_m = psLN.tile([1, 512], F32, tag="st")
                    for kc in range(KC):
                        nc.tensor.matmul(ps_m, lhsT=cm_col, rhs=xt[:, kc],
                                         start=(kc == 0), stop=(kc == KC - 1))
                    m_row = lnR.tile([1, 512], BF16, tag="row", name=f"mrow{n}")
                    with nc.allow_low_precision(reason="LN mean row"):
                        nc.scalar.activation(m_row, ps_m, AF.Copy)
                    m_bc = lnZ.tile([P, 512], BF16, tag="mbc", name=f"mbc{n}")
                    nc.gpsimd.partition_broadcast(m_bc, m_row)
                    ps_v = psLN.tile([1, 512], F32, tag="st")
                    for kc in range(KC):
                        nc.tensor.matmul(ps_v, lhsT=cm_col, rhs=sq_t[:, kc],
                                         start=(kc == 0), stop=(kc == KC - 1))
                    msq_row = lnR.tile([1, 512], F32, tag="row", name=f"msqrow{n}")
                    nc.scalar.square(msq_row, m_row)
                    v_row = lnR.tile([1, 512], F32, tag="row", name=f"vrow{n}")
                    nc.vector.tensor_tensor(out=v_row, in0=ps_v, in1=msq_row,
                                            op=ALU.subtract)
                    zc = lnZ.tile([P, KC, 512], BF16, tag="zc", name=f"zc{n}")
                    nc.vector.tensor_tensor(
                        out=zc, in0=xt,
                        in1=m_bc[:, None, :].to_broadcast([P, KC, 512]),
                        op=ALU.subtract)
                    std_row = lnR.tile([1, 512], F32, tag="row", name=f"sdrow{n}")
                    nc.scalar.activation(std_row, v_row, AF.Sqrt, bias=eps1)
                    rs_row = lnR.tile([1, 512], BF16, tag="row", name=f"rsrow{n}")
                    with nc.allow_low_precision(reason="rstd row"):
                        nc.vector.reciprocal(rs_row, std_row)
                    rs_bc = lnZ.tile([P, 512], BF16, tag="rbc", name=f"rbc{n}")
                    nc.gpsimd.partition_broadcast(rs_bc, rs_row)
                    nc.vector.tensor_tensor(
                        out=z_sb[:, :, sl], in0=zc,
                        in1=rs_bc[:, None, :].to_broadcast([P, KC, 512]),
                        op=ALU.mult)
                    # fp8 copy of z for the q/k/self-fold DoubleRow matmuls;
                    # runs on the otherwise-idle GPSIMD engine.
                    nc.gpsimd.tensor_copy(z8_sb[:, :, sl], z_sb[:, :, sl])

                    # ---- q/k (fp8 DR) and v (bf16) for this token tile -----
                    if n == 0:
                        for mo in range(KC):
                            ps = psMM.tile([P, 512], F32, tag="mm")
                            for kp in range(2):
                                nc.tensor.matmul(
                                    ps,
                                    lhsT=wq_sb[:, 2 * kp:2 * kp + 2,
                                               mo * P:(mo + 1) * P],
                                    rhs=z8_sb[:, 2 * kp:2 * kp + 2, 0:TQ],
                                    start=(kp == 0), stop=(kp == 1),
                                    perf_mode=DR)
                            if mo < 2:
                                nc.scalar.activation(q_sb[:, mo], ps, AF.Identity,
                                                     bias=bq_sb[:, mo:mo + 1])
                            else:
                                nc.vector.tensor_scalar(q_sb[:, mo], ps,
                                                        bq_sb[:, mo:mo + 1],
                                                        None, ALU.add)
                    for mo in range(KC):
                        ps = psMM.tile([P, 512], F32, tag="mm")
                        for kp in range(2):
                            nc.tensor.matmul(
                                ps,
                                lhsT=wk_sb[:, 2 * kp:2 * kp + 2,
                                           mo * P:(mo + 1) * P],
                                rhs=z8_sb[:, 2 * kp:2 * kp + 2, sl],
                                start=(kp == 0), stop=(kp == 1),
                                perf_mode=DR)
                        if mo < 2:
                            nc.scalar.activation(k_sb[:, mo, sl], ps, AF.Identity,
                                                 bias=bk_sb[:, mo:mo + 1])
                        else:
                            nc.vector.tensor_scalar(k_sb[:, mo, sl], ps,
                                                    bk_sb[:, mo:mo + 1],
                                                    None, ALU.add)
                    for ts_ in range(4 * n, 4 * n + 4):
                        ps = psMM.tile([P, 512], F32, tag="mm")
                        for kc in range(KC):
                            nc.tensor.matmul(
                                ps,
                                lhsT=z_sb[:, kc, ts_ * P:(ts_ + 1) * P],
                                rhs=wv_sb[:, kc],
                                start=(kc == 0), stop=False)
                        nc.tensor.matmul(ps, lhsT=onesr_sb, rhs=bv_sb,
                                         start=False, stop=True,
                                         skip_group_check=True)
                        if ts_ % 2 == 0:
                            nc.vector.tensor_copy(v_sb[:, ts_], ps)
                        else:
                            nc.scalar.activation(v_sb[:, ts_], ps, AF.Identity)

        # ======= Attention + per-half tails =================================
        y2 = lateP.tile([P, KC, TQ], BF16, tag="mid_a")
        y3 = lateP.tile([P, KC, TQ], BF16, tag="mid_b")
        z2 = lateP.tile([P, KC, TQ], BF16, tag="z2")

        def attention_pass(hg, hq, psSC, attS, y_ph, filler=None):
            """heads [4*hg, 4*hg+4), queries [hq*HQ, hq*HQ+HQ).

            Each pass holds 6 PSUM banks (scores 2x2, y accumulators 2),
            leaving 2 banks for a concurrently-scheduled tail stage.
            Score PSUM tiles are [P, 2, 512] so each head's accumulation
            group gets a full bank (matmul start zeroes 2KB regions).
            """
            qs = slice(hq * HQ, (hq + 1) * HQ)
            for tkc in range(NTK):
                coul_t = coul_sb[:, tkc, qs]
                s_t = attS.tile([P, 4, HQ], BF16, tag="st")
                for quarter in range(2):
                    sc_ps = psSC.tile([P, 2, 512], F32, tag="sc")
                    for hh in range(2):
                        h = 4 * hg + quarter * 2 + hh
                        chk, po = h // 2, 64 * (h % 2)
                        ck = slice(chk, KC + 1, KC - chk)
                        nc.tensor.matmul(
                            sc_ps[:, hh, 0:HQ],
                            lhsT=k_sb[po:po + 64, ck,
                                      tkc * P:(tkc + 1) * P],
                            rhs=q_sb[po:po + 64, ck, qs],
                            start=True, stop=True, perf_mode=DR)
                    nc.scalar.activation(
                        s_t[:, quarter * 2:quarter * 2 + 2, :],
                        sc_ps[:, :, 0:HQ], AF.Sigmoid, scale=SIGS)
                nc.vector.tensor_tensor(
                    out=s_t, in0=s_t,
                    in1=coul_t[:, None, :].to_broadcast([P, 4, HQ]),
                    op=ALU.mult)
                for hh in range(4):
                    h = 4 * hg + hh
                    po = 64 * (h % 2)
                    nc.tensor.matmul(
                        y_ph[hh // 2][po:po + 64, :],
                        lhsT=v_sb[:, tkc, 64 * h:64 * h + 64],
                        rhs=s_t[:, hh, :],
                        start=(tkc == 0), stop=(tkc == NTK - 1),
                        tile_position=(0, po))
                if filler is not None:
                    for _ in range(2):
                        next(filler, None)

        def tail(hq):
            """proj + LN2 + MLP + out DMA for one query half.

            Uses at most 2 PSUM banks at any time so it can run concurrently
            with an attention pass (6 banks).
            """
            qs = slice(hq * HQ, (hq + 1) * HQ)
            y2h = y2[:, :, qs]
            y3h = y3[:, :, qs]
            z2h = z2[:, :, qs]
            with tc.tile_pool(name=f"psP5{hq}", bufs=2, space="PSUM") as psP5:
                for j in range(KC):
                    ps = psP5.tile([P, HQ], F32, tag="mm")
                    for kc in range(KC):
                        nc.tensor.matmul(
                            ps, lhsT=wproj_sb[:, kc, j * P:(j + 1) * P],
                            rhs=y2h[:, kc],
                            start=(kc == 0), stop=False)
                    for kp in range(2):
                        nc.tensor.matmul(
                            ps,
                            lhsT=wsp_sb[:, 2 * kp:2 * kp + 2, j * P:(j + 1) * P],
                            rhs=z8_sb[:, 2 * kp:2 * kp + 2, qs],
                            start=False, stop=(kp == 1), perf_mode=DR,
                            skip_group_check=True)
                    if j % 2 == 0:
                        nc.vector.tensor_scalar(y3h[:, j], ps, IWS,
                                                bproj_sb[:, j:j + 1], ALU.mult,
                                                ALU.add)
                    else:
                        nc.scalar.activation(y3h[:, j], ps, AF.Identity,
                                             bias=bproj_sb[:, j:j + 1],
                                             scale=IWS)
                    yield

            with tc.tile_pool(name=f"ln2R{hq}", bufs=6) as ln2R, \
                 tc.tile_pool(name=f"ln2S{hq}", bufs=1) as ln2S, \
                 tc.tile_pool(name=f"ln2T{hq}", bufs=2) as ln2T, \
                 tc.tile_pool(name=f"psL2{hq}", bufs=1, space="PSUM") as psLN2:
                sq2 = ln2S.tile([P, KC, HQ], BF16, tag="sq2")
                nc.vector.tensor_tensor(out=sq2, in0=y3h, in1=y3h, op=ALU.mult)
                ps_m2 = psLN2.tile([1, HQ], F32, tag="st2")
                for kc in range(KC):
                    nc.tensor.matmul(ps_m2, lhsT=cm_col, rhs=y3h[:, kc],
                                     start=(kc == 0), stop=(kc == KC - 1))
                m2_row = ln2R.tile([1, HQ], BF16, tag="row2")
                with nc.allow_low_precision(reason="LN mean row"):
                    nc.vector.tensor_copy(m2_row, ps_m2)
                m2_bc = ln2T.tile([P, HQ], BF16, tag="m2bc")
                nc.gpsimd.partition_broadcast(m2_bc, m2_row)
                yield
                ps_v2 = psLN2.tile([1, HQ], F32, tag="st2")
                for kc in range(KC):
                    nc.tensor.matmul(ps_v2, lhsT=cm_col, rhs=sq2[:, kc],
                                     start=(kc == 0), stop=(kc == KC - 1))
                msq2_row = ln2R.tile([1, HQ], F32, tag="row2")
                nc.scalar.square(msq2_row, m2_row)
                v2_row = ln2R.tile([1, HQ], F32, tag="row2")
                nc.vector.tensor_tensor(out=v2_row, in0=ps_v2, in1=msq2_row,
                                        op=ALU.subtract)
                zc2 = ln2T.tile([P, KC, HQ], BF16, tag="zc2")
                nc.vector.tensor_tensor(
                    out=zc2, in0=y3h,
                    in1=m2_bc[:, None, :].to_broadcast([P, KC, HQ]),
                    op=ALU.subtract)
                yield
                sd2_row = ln2R.tile([1, HQ], F32, tag="row2")
                nc.scalar.activation(sd2_row, v2_row, AF.Sqrt, bias=eps1)
                rs2_row = ln2R.tile([1, HQ], BF16, tag="row2")
                with nc.allow_low_precision(reason="rstd row"):
                    nc.vector.reciprocal(rs2_row, sd2_row)
                rs2_bc = ln2T.tile([P, HQ], BF16, tag="r2bc")
                nc.gpsimd.partition_broadcast(rs2_bc, rs2_row)
                nc.vector.tensor_tensor(
                    out=z2h, in0=zc2,
                    in1=rs2_bc[:, None, :].to_broadcast([P, KC, HQ]),
                    op=ALU.mult)
                yield

            with tc.tile_pool(name=f"gP{hq}", bufs=1) as gP, \
                 tc.tile_pool(name=f"mT{hq}", bufs=4) as mT:
                g_sb = gP.tile([P, KC4, HQ], BF16)
                out_sb = gP.tile([P, KC, HQ], F32)
                with tc.tile_pool(name=f"psML{hq}", bufs=2,
                                  space="PSUM") as psMLP:
                    for mo in range(KC4):
                        ps = psMLP.tile([P, HQ], F32, tag="mm")
                        for kc in range(KC):
                            nc.tensor.matmul(
                                ps, lhsT=wfc_sb[:, kc, mo * P:(mo + 1) * P],
                                rhs=z2h[:, kc],
                                start=(kc == 0), stop=(kc == KC - 1))
                        if hq == 0:
                            # erf-based gelu: erf shares the sigmoid act
                            # table, so this interleaves with attention
                            # passes without table reloads.
                            e_t = mT.tile([P, HQ], BF16, tag="et")
                            nc.scalar.activation(e_t, ps, AF.Erf,
                                                 bias=bfce_sb[:, mo:mo + 1],
                                                 scale=0.7071067811865476)
                            t_t = mT.tile([P, HQ], BF16, tag="tt")
                            nc.vector.tensor_scalar(t_t, ps, 0.5,
                                                    bfch_sb[:, mo:mo + 1],
                                                    ALU.mult, ALU.add)
                            nc.vector.scalar_tensor_tensor(
                                g_sb[:, mo], e_t, 1.0, t_t,
                                ALU.add, ALU.mult)
                        else:
                            nc.scalar.activation(g_sb[:, mo], ps, AF.Gelu,
                                                 bias=bfc_sb[:, mo:mo + 1])
                        if mo % 2 == 1:
                            yield
                with tc.tile_pool(name=f"psOJ{hq}", bufs=2,
                                  space="PSUM") as psOJ:
                    for j in range(KC):
                        oj = psOJ.tile([P, HQ], F32, tag="oj")
                        for mo in range(KC4):
                            nc.tensor.matmul(
                                oj, lhsT=wfcp_sb[:, mo, j * P:(j + 1) * P],
                                rhs=g_sb[:, mo],
                                start=(mo == 0), stop=(mo == KC4 - 1))
                        if j % 2 == 0:
                            nc.vector.tensor_scalar(out_sb[:, j], oj,
                                                    bfcp_sb[:, j:j + 1], None,
                                                    ALU.add)
                        else:
                            nc.scalar.activation(out_sb[:, j], oj, AF.Identity,
                                                 bias=bfcp_sb[:, j:j + 1])
                        nc.sync.dma_start(outT_d[:, j, qs], out_sb[:, j])
                        yield

        # Pass order: (heads 0-3, qA), (heads 4-7, qA), (heads 0-3, qB),
        # (heads 4-7, qB). tail(qA) is emitted before the last pass so the
        # scheduler runs it under pass 4's sigmoid wall; only tail(qB) is
        # exposed at the end.
        with tc.tile_pool(name="attS", bufs=3) as attS, \
             tc.tile_pool(name="psSC", bufs=2, space="PSUM") as psSC:
            def one_pass(hg, hq, filler=None):
                qs = slice(hq * HQ, (hq + 1) * HQ)
                with tc.tile_pool(name=f"psY{hg}{hq}", bufs=1,
                                  space="PSUM") as psY:
                    y_ph = [psY.tile([P, HQ], F32, tag=f"y{jj}",
                                     name=f"y_{hg}{hq}_{jj}")
                            for jj in range(2)]
                    attention_pass(hg, hq, psSC, attS, y_ph, filler=filler)
                    for jj in range(2):
                        j = 2 * hg + jj
                        if jj == 0:
                            nc.vector.tensor_copy(y2[:, j, qs], y_ph[jj])
                        else:
                            nc.scalar.activation(y2[:, j, qs], y_ph[jj],
                                                 AF.Identity)

            one_pass(0, 0)
            one_pass(1, 0)
            tail_a = tail(0)
            one_pass(0, 1, filler=tail_a)
            one_pass(1, 1, filler=tail_a)
            for _ in tail_a:
                pass
        for _ in tail(1):
            pass

    nc.compile()
    return nc


def _get_nc():
    global _BUILT
    if _BUILT is None:
        _BUILT = _build()
    return _BUILT


def _fmt_lhs(w):
    """[Cin, Cout] -> [128, Cin//128, Cout] partition-major lhsT layout."""
    return np.ascontiguousarray(
        w.reshape(w.shape[0] // P, P, w.shape[1]).transpose(1, 0, 2))


def _fmt_bias(b):
    """[O] -> [128, O//128] per-partition layout."""
    return np.ascontiguousarray(b.reshape(-1, P).T)


def _prep(inputs):
    f32 = np.float32
    x = np.asarray(inputs["x"], f32)
    coul = np.asarray(inputs["coulomb_matrix"], f32)
    g1 = np.asarray(inputs["ln1_g"], f32)
    b1 = np.asarray(inputs["ln1_b"], f32)
    g2 = np.asarray(inputs["ln2_g"], f32)
    b2 = np.asarray(inputs["ln2_b"], f32)
    wattn = np.asarray(inputs["w_attn"], f32)
    battn = np.asarray(inputs["b_attn"], f32)
    w_self = np.asarray(inputs["w_self"], f32)
    b_self = np.asarray(inputs["b_self"], f32)
    w_proj = np.asarray(inputs["w_proj"], f32)
    b_proj = np.asarray(inputs["b_proj"], f32)
    w_fc = np.asarray(inputs["w_fc"], f32)
    b_fc = np.asarray(inputs["b_fc"], f32)
    w_fcp = np.asarray(inputs["w_fc_proj"], f32)
    b_fcp = np.asarray(inputs["b_fc_proj"], f32)

    wq, wk, wv = wattn[:, 0:C], wattn[:, C:2 * C], wattn[:, 2 * C:]
    # self branch folded into proj: y3 = att@wproj + z@(g1*wself@wproj) + bp'
    wsp = (g1[:, None] * w_self) @ w_proj
    bproj_f = (b_self + b1 @ w_self) @ w_proj + b_proj

    shared = {
        "wq": _fmt_lhs(g1[:, None] * wq * WS).astype(NP8),
        "wk": _fmt_lhs(g1[:, None] * wk * WS).astype(NP8),
        "wsp": _fmt_lhs(wsp * WS).astype(NP8),
        "wv": _fmt_lhs(g1[:, None] * wv).astype(ml_dtypes.bfloat16),
        "wproj": _fmt_lhs(w_proj * WS).astype(ml_dtypes.bfloat16),
        "wfc": _fmt_lhs(g2[:, None] * w_fc).astype(ml_dtypes.bfloat16),
        "wfcp": _fmt_lhs(w_fcp).astype(ml_dtypes.bfloat16),
        "bq": _fmt_bias((battn[0:C] + b1 @ wq) * WS),
        "bk": _fmt_bias((battn[C:2 * C] + b1 @ wk) * WS),
        "bv": (battn[2 * C:] + b1 @ wv).reshape(1, C),
        "bproj": _fmt_bias(bproj_f),
        "bfc": _fmt_bias(b_fc + b2 @ w_fc),
        "bfce": _fmt_bias((b_fc + b2 @ w_fc) * 0.7071067811865476),
        "bfch": _fmt_bias((b_fc + b2 @ w_fc) * 0.5),
        "bfcp": _fmt_bias(b_fcp),
        "cst": np.stack([np.ones(P, f32), np.full(P, 1.0 / C, f32)],
                        axis=1).astype(ml_dtypes.bfloat16),
        "onesr": np.ones((1, P), f32),
    }
    in_maps = []
    for core in range(N_CORES):
        b, tqi = divmod(core, 4)
        tq0 = tqi * TQ
        xr = np.roll(x[b], -tq0, axis=0)                      # [T, C]
        xT = np.ascontiguousarray(
            xr.T.reshape(KC, P, T).transpose(1, 0, 2)).astype(
                ml_dtypes.bfloat16)                           # [P, KC, T]
        cr = np.roll(coul[b], -tq0, axis=1)[tq0:tq0 + TQ, :]  # [TQ, T]
        coulT = np.ascontiguousarray(
            cr.T.reshape(NTK, P, TQ).transpose(1, 0, 2)).astype(
                ml_dtypes.bfloat16)                           # [P, NTK, TQ]
        m = dict(shared)
        m["xT"] = xT
        m["coulT"] = coulT
        in_maps.append(m)
    return in_maps


def _assemble(results):
    out = np.empty((B, T, C), np.float32)
    for core in range(N_CORES):
        b, tqi = divmod(core, 4)
        tq0 = tqi * TQ
        r = results[core]["outT"]                  # [P, KC, TQ]
        o = r.transpose(1, 0, 2).reshape(C, TQ).T  # [TQ, C]
        out[b, tq0:tq0 + TQ] = o
    return out


def _run(inputs, trace=False):
    nc = _get_nc()
    in_maps = _prep(inputs)
    res = run_bass_kernel_spmd(nc, in_maps, core_ids=list(range(N_CORES)),
                               trace=trace)
    return _assemble(res.results), res


def kernel(**inputs):
    out, _ = _run(inputs)
    return out
